# revision 17
# baseline (speedup 1.0000x reference)
"""Trainium2 Bass kernel for the correlation-map embedding module (v14).

Math (per (b, nf) pair):
  f1d = bilinear_down28(feature_i[b, nf])                  # [C, 28, 28]
  f2sel[c, k] = bilinear sample of feature_j[b, nf] at the K knn grid points
  corr[k, :, :] = relu(sum_c f2sel[c, k] * f1d[c, :, :])   # [K, 28, 28]
  out[k] = corr[k] / sum_hw(exp(corr[k])) * 10

Structure (lineage: v8 host-gathered taps 61.6us, v10 spread epilogue +
early fjg 50.0us):
  - feature_j's knn tap rows are gathered on the HOST (knn_inds is a
    kernel input; the host already repacks/casts everything) into 1024B
    rows [j, (pos, b, c)] fp16; the device loads 768KB of tap rows
    instead of 9.6MB of fj.
  - f2sel[c,k] = sum_j g[j,c]*Wsel[j,k] on the PE: 4 accumulating
    128x128 matmuls per pair against a host-built block-sparse weight
    matrix (f32 PSUM), then one ScalarE copy to fp16 SBUF. All 6 pairs
    run up-front at ~12us - they only need the small const load.
  - feature_i arrives fp16 host-deinterleaved into tap-plane order
    [NF, BPC, C, (u,t,gh,gw)]: the 4-tap downsample weighting is ONE
    contiguous DVE fp16 2x multiply per batch, the corr matmul's
    moving operand slices are fully CONTIGUOUS tap planes (a strided
    rhs cost ~+200ns per matmul in v11), and the tap summation rides
    the PSUM accumulation. fi loads are split per batch so the first
    corr matmuls start one load earlier.
  - all weight constants arrive pre-broadcast/pre-built in ONE [128,
    7744] fp16 DMA (wsel | w4il | tap rows): no PE ones-broadcasts.
  - engine-phase program order prevents FIFO head-of-line blocking:
    all six DVE tap-muls are emitted before any epilogue DVE op, so a
    later nf's tap-mul never queues behind an earlier nf's epilogue.
  - epilogue: relu+exp(+accum) on ScalarE reading PSUM, reciprocal and
    the normalize multiply on DVE, stores issued from the Sync queue.
    (GPSIMD measured ~12us per 784-elem op + DVE port contention, so
    it gets no elementwise work.)

Sharding: pure data parallel - batch dim (16) split across 8 cores, 2 each.
"""

import numpy as np

# hardcoded problem shapes (grading calls kernel(**inputs) standalone)
B, NF, C, H, W = 16, 3, 128, 56, 56
G = 28
K = 128
NCORES = 8
BPC = B // NCORES  # 2
P = 128
QH = G * G // 2  # 392 psum columns per bank
GH = G // 2
NIDX = K * 2  # 256 gather rows per nf (column-pair rows, j = k*2 + u)
RB = 2 * BPC * C  # 512 f16 per tap row: (pos, b, c)
NWSEL = NF * 4 * K  # 1536
NW4 = 4 * G * G  # 3136
NFJG = NF * 2 * RB  # 3072
NCOMBO = NWSEL + NW4 + NFJG  # 7744 f16 per partition

_CACHE = {}


def _axis_coords(n_in):
    # float32 arithmetic to match the jax reference bit-for-bit
    src = np.arange(G, dtype=np.float32) * np.float32((n_in - 1) / (G - 1))
    i0 = np.clip(np.floor(src).astype(np.int32), 0, n_in - 2)
    w = (src - i0.astype(np.float32)).astype(np.float32)
    return i0, w


def _host_consts(knn_inds):
    i0h, wh = _axis_coords(H)
    i0w, ww = _axis_coords(W)
    # the even/odd strided-AP downsample assumes taps are (2k, 2k+1)
    assert np.array_equal(i0h, 2 * np.arange(G)) and np.array_equal(i0w, 2 * np.arange(G))

    ah, bh = (1.0 - wh), wh
    aw, bw = (1.0 - ww), ww
    # tap-plane (u, t, gh, gw) order matching the host-deinterleaved f1
    w4il = np.stack(
        [np.outer(ah, aw), np.outer(ah, bw), np.outer(bh, aw), np.outer(bh, bw)]
    ).reshape(-1).astype(np.float16)  # [4*784]

    knn = np.asarray(knn_inds).astype(np.int64)  # [NF, K, 2]
    rows_all = []
    # block-sparse tap-weight matrices: f2sel[c,k] = sum_j g[j,c]*Wsel[j,k];
    # j = k*2 + u, chunk s covers j in [128s, 128s+128) (partition p = j-128s),
    # pos = W-axis tap t. Layout [P, NF, s, pos, K].
    wsel = np.zeros((P, NF, 2, 2, K), dtype=np.float16)
    for nf in range(NF):
        h2 = knn[nf, :, 1]
        w2 = knn[nf, :, 0]
        r0 = i0h[h2]
        c0 = i0w[w2]
        rows = np.stack(
            [r0 * (W // 2) + c0 // 2, (r0 + 1) * (W // 2) + c0 // 2], axis=1
        ).reshape(-1)  # [256], j = k*2 + u
        rows_all.append(rows)
        wu = np.stack([ah[h2], bh[h2]], axis=1).reshape(-1)  # [256] per (k,u)
        wt = np.stack([aw[w2], bw[w2]], axis=1)  # [K, 2] per (k,t)
        for s_ in range(2):
            for p in range(128):
                j = 128 * s_ + p
                k = j // 2
                wsel[p, nf, s_, 0, k] = wu[j] * wt[k, 0]
                wsel[p, nf, s_, 1, k] = wu[j] * wt[k, 1]
    return w4il, wsel, rows_all


def _build_bass():
    import concourse.bacc as bacc
    import concourse.tile as tile
    from concourse import mybir

    f32 = mybir.dt.float32
    f16 = mybir.dt.float16
    AF = mybir.ActivationFunctionType

    nc = bacc.Bacc()
    fi = nc.dram_tensor("fi", [NF, BPC, C, H * W], f16, kind="ExternalInput")
    combo_d = nc.dram_tensor("combo", [P, NCOMBO], f16, kind="ExternalInput")
    out_d = nc.dram_tensor("out", [NF, BPC, K, G * G], f32, kind="ExternalOutput")

    with tile.TileContext(nc) as tc:
        with (
            tc.tile_pool(name="consts", bufs=1) as consts,
            tc.tile_pool(name="feat1", bufs=1) as feat1,
            tc.tile_pool(name="work", bufs=3) as work,
            tc.tile_pool(name="sel", bufs=1) as selp,
            tc.tile_pool(name="psum", bufs=3, space="PSUM") as pspool,
            tc.tile_pool(name="fsel", bufs=2, space="PSUM") as fselpool,
            tc.tile_pool(name="outp", bufs=4) as outp,
        ):
            # ---- loads: consts first (f2sel only needs these), then fi ----
            combo = consts.tile([P, NCOMBO], f16, tag="combo")
            nc.sync.dma_start(out=combo, in_=combo_d[:, :])
            wsel_t = combo[:, :NWSEL].rearrange(
                "p (a b c d) -> p a b c d", a=NF, b=2, c=2
            )
            w4il_t = combo[:, NWSEL : NWSEL + NW4]
            g2a = combo[:, NWSEL + NW4 :].rearrange(
                "p (a b c) -> p a b c", a=NF, b=2
            )

            f1xs = []
            for nf in range(NF):
                t = feat1.tile([P, BPC, H * W], f16, tag=f"f1x{nf}")
                for b in range(BPC):
                    nc.sync.dma_start(out=t[:, b], in_=fi[nf, b])
                f1xs.append(t)

            # ---- phase A: all six f2sel = g.T @ Wsel (PE) + fp16 copies ----
            f2sels = {}
            for nf in range(NF):
                gv = g2a[:, nf].rearrange(
                    "p s (pos b c) -> p s pos b c", pos=2, b=BPC
                )
                for b in range(BPC):
                    fps = fselpool.tile([P, 512], f32, tag="fps")
                    n4 = 0
                    for s_ in range(2):
                        for pos in range(2):
                            nc.tensor.matmul(
                                fps[:, :K],
                                lhsT=gv[:, s_, pos, b],
                                rhs=wsel_t[:, nf, s_, pos],
                                start=(n4 == 0),
                                stop=(n4 == 3),
                            )
                            n4 += 1
                    f2sel = selp.tile([P, K], f16, tag=f"f2sel{nf}{b}")
                    nc.scalar.copy(f2sel, fps[:, :K])
                    f2sels[(nf, b)] = f2sel

            # ---- phase B: all six tap-weight multiplies (DVE fp16 2x) ----
            ms = {}
            for nf in range(NF):
                for b in range(BPC):
                    ma = work.tile([P, H * W], f16, tag=f"ma{b}")
                    nc.vector.tensor_mul(ma, f1xs[nf][:, b], w4il_t)
                    ms[(nf, b)] = ma.rearrange("p (u q) -> p u q", u=4)

            # ---- phase C: corr matmuls + epilogue per pair ----
            for nf in range(NF):
                o2 = outp.tile([P, BPC, G * G], f32, tag="o2")
                for b in range(BPC):
                    # corr[k, q] = sum_c f2sel[c,k] * sum_u m_u[c,q]
                    ps = pspool.tile([P, 2, 512], f32, tag="ps")
                    for half in range(2):
                        lo = half * QH
                        for u4 in range(4):
                            nc.tensor.matmul(
                                ps[:, half, :QH],
                                lhsT=f2sels[(nf, b)],
                                rhs=ms[(nf, b)][:, u4, lo : lo + QH],
                                start=(u4 == 0),
                                stop=(u4 == 3),
                            )

                    # r = 10*relu(corr); s = sum(exp(r/10)); out = r*(1/s)
                    r = outp.tile([P, 2, QH], f32, tag="r")
                    nc.scalar.activation(r, ps[:, :, :QH], AF.Relu, scale=10.0)
                    rf = r.rearrange("p h q -> p (h q)")
                    e = work.tile([P, G * G], f32, tag="e")
                    s = work.tile([P, 1], f32, tag="s")
                    nc.scalar.activation(e, rf, AF.Exp, scale=0.1, accum_out=s)
                    rec = work.tile([P, 1], f32, tag="rec")
                    nc.vector.reciprocal(rec, s)
                    nc.vector.tensor_scalar(
                        o2[:, b], rf, rec, None, op0=mybir.AluOpType.mult
                    )
                    # store from the post-load-idle Sync queue
                    nc.sync.dma_start(out=out_d[nf, b], in_=o2[:, b])
    return nc


def _get_bass():
    if "nc" not in _CACHE:
        nc = _build_bass()
        if not nc.is_finalized():
            nc.finalize()
        _CACHE["nc"] = nc
    return _CACHE["nc"]


def _prepare_in_maps(feature_i, feature_j, knn_inds):
    w4il, wsel, rows_all = _host_consts(knn_inds)
    fi = np.asarray(feature_i, dtype=np.float32).reshape(
        NCORES, BPC, NF, C, G, 2, G, 2
    )
    # [core,b,nf,c,gh,u,gw,t] -> [core, nf, b, c, u, t, gh, gw] fp16:
    # tap-plane order makes both the DVE weighting and the corr matmul
    # moving operand fully contiguous
    fi = np.ascontiguousarray(fi.transpose(0, 2, 1, 3, 5, 7, 4, 6)).astype(np.float16)
    fi = fi.reshape(NCORES, NF, BPC, C, H * W)
    fj = np.asarray(feature_j, dtype=np.float32).reshape(
        NCORES, BPC, NF, C, H, W // 2, 2
    )
    # [core,b,nf,c,h,wp,pos] -> [core, nf, (h wp), pos, b, c] fp16 rows,
    # then host-gather the knn tap rows: [core, nf, j(256), (pos, b, c)]
    fjt = np.ascontiguousarray(fj.transpose(0, 2, 4, 5, 6, 1, 3)).astype(np.float16)
    fjt = fjt.reshape(NCORES, NF, H * W // 2, RB)
    fjg = np.empty((NCORES, NF, NIDX, RB), dtype=np.float16)
    for nf in range(NF):
        fjg[:, nf] = fjt[:, nf, rows_all[nf]]
    # row j -> partition j%128, slot j//128: [core, P, nf, s, RB]
    fjg = fjg.reshape(NCORES, NF, 2, P, RB).transpose(0, 3, 1, 2, 4)

    combo = np.concatenate(
        [
            wsel.reshape(P, NWSEL),
            np.broadcast_to(w4il[None, :], (P, NW4)),
            np.ascontiguousarray(fjg).reshape(NCORES, P, NFJG).transpose(1, 0, 2)[
                :, 0, :
            ]
            * 0,  # placeholder, per-core below
        ],
        axis=1,
    ).astype(np.float16)
    fjg_flat = np.ascontiguousarray(fjg).reshape(NCORES, P, NFJG)

    in_maps = []
    for core in range(NCORES):
        cb = combo.copy()
        cb[:, NWSEL + NW4 :] = fjg_flat[core]
        in_maps.append({"fi": fi[core], "combo": cb})
    return in_maps


def kernel(feature_i, feature_j, mask, optical_flow, knn_inds):
    from concourse import bass_utils

    nc = _get_bass()
    in_maps = _prepare_in_maps(feature_i, feature_j, knn_inds)

    res = bass_utils.run_bass_kernel_spmd(nc, in_maps, core_ids=list(range(NCORES)))
    out = np.stack([res.results[c]["out"] for c in range(NCORES)], axis=0)
    out = out.reshape(NCORES, NF, BPC, K, G, G).transpose(0, 2, 1, 3, 4, 5)
    return np.ascontiguousarray(out.reshape(B, NF, K, G, G)).astype(np.float32)


# revision 18
# speedup vs baseline: 1.0253x; 1.0253x over previous
"""Trainium2 Bass kernel for the correlation-map embedding module (v12).

Math (per (b, nf) pair):
  f1d = bilinear_down28(feature_i[b, nf])                  # [C, 28, 28]
  f2sel[c, k] = bilinear sample of feature_j[b, nf] at the K knn grid points
  corr[k, :, :] = relu(sum_c f2sel[c, k] * f1d[c, :, :])   # [K, 28, 28]
  out[k] = corr[k] / sum_hw(exp(corr[k])) * 10

Structure (lineage: v8 host-gathered taps 61.6us, v10 spread epilogue +
early fjg 50.0us):
  - feature_j's knn tap rows are gathered on the HOST (knn_inds is a
    kernel input; the host already repacks/casts everything) into 1024B
    rows [j, (pos, b, c)] fp16; the device loads 768KB of tap rows
    instead of 9.6MB of fj.
  - f2sel[c,k] = sum_j g[j,c]*Wsel[j,k] on the PE: 4 accumulating
    128x128 matmuls per pair against a host-built block-sparse weight
    matrix (f32 PSUM), then one ScalarE copy to fp16 SBUF. All 6 pairs
    run up-front at ~12us - they only need the small const load.
  - feature_i arrives fp16 host-deinterleaved into tap-plane order
    [NF, BPC, C, (u,t,gh,gw)]: the 4-tap downsample weighting is ONE
    contiguous DVE fp16 2x multiply per batch, the corr matmul's
    moving operand slices are fully CONTIGUOUS tap planes (a strided
    rhs cost ~+200ns per matmul in v11), and the tap summation rides
    the PSUM accumulation. fi loads are split per batch so the first
    corr matmuls start one load earlier.
  - all weight constants arrive pre-broadcast/pre-built in ONE [128,
    7744] fp16 DMA (wsel | w4il | tap rows): no PE ones-broadcasts.
  - engine-phase program order prevents FIFO head-of-line blocking:
    all six DVE tap-muls are emitted before any epilogue DVE op, so a
    later nf's tap-mul never queues behind an earlier nf's epilogue.
  - epilogue: relu+exp(+accum) on ScalarE reading PSUM, reciprocal and
    the normalize multiply on DVE, stores issued from the Sync queue.
    (GPSIMD measured ~12us per 784-elem op + DVE port contention, so
    it gets no elementwise work.)

Sharding: pure data parallel - batch dim (16) split across 8 cores, 2 each.
"""

import numpy as np

# hardcoded problem shapes (grading calls kernel(**inputs) standalone)
B, NF, C, H, W = 16, 3, 128, 56, 56
G = 28
K = 128
NCORES = 8
BPC = B // NCORES  # 2
P = 128
QH = G * G // 2  # 392 psum columns per bank
GH = G // 2
NIDX = K * 2  # 256 gather rows per nf (column-pair rows, j = k*2 + u)
RB = 2 * BPC * C  # 512 f16 per tap row: (pos, b, c)
NWSEL = NF * 4 * K  # 1536
NW4 = 4 * G * G  # 3136
NFJG = NF * 2 * RB  # 3072
NCOMBO = NWSEL + NW4 + NFJG  # 7744 f16 per partition

_CACHE = {}


def _axis_coords(n_in):
    # float32 arithmetic to match the jax reference bit-for-bit
    src = np.arange(G, dtype=np.float32) * np.float32((n_in - 1) / (G - 1))
    i0 = np.clip(np.floor(src).astype(np.int32), 0, n_in - 2)
    w = (src - i0.astype(np.float32)).astype(np.float32)
    return i0, w


def _host_consts(knn_inds):
    i0h, wh = _axis_coords(H)
    i0w, ww = _axis_coords(W)
    # the even/odd strided-AP downsample assumes taps are (2k, 2k+1)
    assert np.array_equal(i0h, 2 * np.arange(G)) and np.array_equal(i0w, 2 * np.arange(G))

    ah, bh = (1.0 - wh), wh
    aw, bw = (1.0 - ww), ww
    # tap-plane (u, t, gh, gw) order matching the host-deinterleaved f1
    w4il = np.stack(
        [np.outer(ah, aw), np.outer(ah, bw), np.outer(bh, aw), np.outer(bh, bw)]
    ).reshape(-1).astype(np.float16)  # [4*784]

    knn = np.asarray(knn_inds).astype(np.int64)  # [NF, K, 2]
    rows_all = []
    # block-sparse tap-weight matrices: f2sel[c,k] = sum_j g[j,c]*Wsel[j,k];
    # j = k*2 + u, chunk s covers j in [128s, 128s+128) (partition p = j-128s),
    # pos = W-axis tap t. Layout [P, NF, s, pos, K].
    wsel = np.zeros((P, NF, 2, 2, K), dtype=np.float16)
    for nf in range(NF):
        h2 = knn[nf, :, 1]
        w2 = knn[nf, :, 0]
        r0 = i0h[h2]
        c0 = i0w[w2]
        rows = np.stack(
            [r0 * (W // 2) + c0 // 2, (r0 + 1) * (W // 2) + c0 // 2], axis=1
        ).reshape(-1)  # [256], j = k*2 + u
        rows_all.append(rows)
        wu = np.stack([ah[h2], bh[h2]], axis=1).reshape(-1)  # [256] per (k,u)
        wt = np.stack([aw[w2], bw[w2]], axis=1)  # [K, 2] per (k,t)
        for s_ in range(2):
            for p in range(128):
                j = 128 * s_ + p
                k = j // 2
                wsel[p, nf, s_, 0, k] = wu[j] * wt[k, 0]
                wsel[p, nf, s_, 1, k] = wu[j] * wt[k, 1]
    return w4il, wsel, rows_all


def _build_bass():
    import concourse.bacc as bacc
    import concourse.tile as tile
    from concourse import mybir

    f32 = mybir.dt.float32
    f16 = mybir.dt.float16
    AF = mybir.ActivationFunctionType

    nc = bacc.Bacc()
    fi = nc.dram_tensor("fi", [NF, BPC, C, H * W], f16, kind="ExternalInput")
    combo_d = nc.dram_tensor("combo", [P, NCOMBO], f16, kind="ExternalInput")
    out_d = nc.dram_tensor("out", [NF, BPC, K, G * G], f32, kind="ExternalOutput")

    with tile.TileContext(nc) as tc:
        with (
            tc.tile_pool(name="consts", bufs=1) as consts,
            tc.tile_pool(name="feat1", bufs=1) as feat1,
            tc.tile_pool(name="work", bufs=2) as work,
            tc.tile_pool(name="sel", bufs=1) as selp,
            tc.tile_pool(name="psum", bufs=2, space="PSUM") as pspool,
            tc.tile_pool(name="fsel", bufs=2, space="PSUM") as fselpool,
            tc.tile_pool(name="outp", bufs=3) as outp,
        ):
            # ---- loads: consts first (f2sel only needs these), then fi ----
            combo = consts.tile([P, NCOMBO], f16, tag="combo")
            nc.sync.dma_start(out=combo, in_=combo_d[:, :])
            wsel_t = combo[:, :NWSEL].rearrange(
                "p (a b c d) -> p a b c d", a=NF, b=2, c=2
            )
            w4il_t = combo[:, NWSEL : NWSEL + NW4]
            g2a = combo[:, NWSEL + NW4 :].rearrange(
                "p (a b c) -> p a b c", a=NF, b=2
            )

            f1xs = []
            for nf in range(NF):
                t = feat1.tile([P, BPC, H * W], f16, tag=f"f1x{nf}")
                for b in range(BPC):
                    nc.sync.dma_start(out=t[:, b], in_=fi[nf, b])
                f1xs.append(t)

            # ---- phase A: all six f2sel = g.T @ Wsel (PE) + fp16 copies ----
            f2sels = {}
            for nf in range(NF):
                gv = g2a[:, nf].rearrange(
                    "p s (pos b c) -> p s pos b c", pos=2, b=BPC
                )
                for b in range(BPC):
                    fps = fselpool.tile([P, 512], f32, tag="fps")
                    n4 = 0
                    for s_ in range(2):
                        for pos in range(2):
                            nc.tensor.matmul(
                                fps[:, :K],
                                lhsT=gv[:, s_, pos, b],
                                rhs=wsel_t[:, nf, s_, pos],
                                start=(n4 == 0),
                                stop=(n4 == 3),
                            )
                            n4 += 1
                    f2sel = selp.tile([P, K], f16, tag=f"f2sel{nf}{b}")
                    nc.scalar.copy(f2sel, fps[:, :K])
                    f2sels[(nf, b)] = f2sel

            # ---- phase B: all six tap-weight multiplies (DVE fp16 2x) ----
            ms = {}
            for nf in range(NF):
                for b in range(BPC):
                    ma = work.tile([P, H * W], f16, tag=f"ma{b}")
                    nc.vector.tensor_mul(ma, f1xs[nf][:, b], w4il_t)
                    ms[(nf, b)] = ma.rearrange("p (u q) -> p u q", u=4)

            # ---- phase C: corr matmuls + epilogue per pair ----
            for nf in range(NF):
                o2 = outp.tile([P, BPC, G * G], f32, tag="o2")
                for b in range(BPC):
                    # corr[k, q] = sum_c f2sel[c,k] * sum_u m_u[c,q]
                    ps = pspool.tile([P, 2, 512], f32, tag="ps")
                    for half in range(2):
                        lo = half * QH
                        for u4 in range(4):
                            nc.tensor.matmul(
                                ps[:, half, :QH],
                                lhsT=f2sels[(nf, b)],
                                rhs=ms[(nf, b)][:, u4, lo : lo + QH],
                                start=(u4 == 0),
                                stop=(u4 == 3),
                            )

                    # r = 10*relu(corr); s = sum(exp(r/10)); out = r*(1/s)
                    r = outp.tile([P, 2, QH], f32, tag="r")
                    nc.scalar.activation(r, ps[:, :, :QH], AF.Relu, scale=10.0)
                    rf = r.rearrange("p h q -> p (h q)")
                    e = work.tile([P, G * G], f32, tag="e")
                    s = work.tile([P, 1], f32, tag="s")
                    nc.scalar.activation(e, rf, AF.Exp, scale=0.1, accum_out=s)
                    rec = work.tile([P, 1], f32, tag="rec")
                    nc.vector.reciprocal(rec, s)
                    nc.vector.tensor_scalar(
                        o2[:, b], rf, rec, None, op0=mybir.AluOpType.mult
                    )
                    # store from the post-load-idle Sync queue
                    nc.sync.dma_start(out=out_d[nf, b], in_=o2[:, b])
    return nc


def _get_bass():
    if "nc" not in _CACHE:
        nc = _build_bass()
        if not nc.is_finalized():
            nc.finalize()
        _CACHE["nc"] = nc
    return _CACHE["nc"]


def _prepare_in_maps(feature_i, feature_j, knn_inds):
    w4il, wsel, rows_all = _host_consts(knn_inds)
    fi = np.asarray(feature_i, dtype=np.float32).reshape(
        NCORES, BPC, NF, C, G, 2, G, 2
    )
    # [core,b,nf,c,gh,u,gw,t] -> [core, nf, b, c, u, t, gh, gw] fp16:
    # tap-plane order makes both the DVE weighting and the corr matmul
    # moving operand fully contiguous
    fi = np.ascontiguousarray(fi.transpose(0, 2, 1, 3, 5, 7, 4, 6)).astype(np.float16)
    fi = fi.reshape(NCORES, NF, BPC, C, H * W)
    fj = np.asarray(feature_j, dtype=np.float32).reshape(
        NCORES, BPC, NF, C, H, W // 2, 2
    )
    # [core,b,nf,c,h,wp,pos] -> [core, nf, (h wp), pos, b, c] fp16 rows,
    # then host-gather the knn tap rows: [core, nf, j(256), (pos, b, c)]
    fjt = np.ascontiguousarray(fj.transpose(0, 2, 4, 5, 6, 1, 3)).astype(np.float16)
    fjt = fjt.reshape(NCORES, NF, H * W // 2, RB)
    fjg = np.empty((NCORES, NF, NIDX, RB), dtype=np.float16)
    for nf in range(NF):
        fjg[:, nf] = fjt[:, nf, rows_all[nf]]
    # row j -> partition j%128, slot j//128: [core, P, nf, s, RB]
    fjg = fjg.reshape(NCORES, NF, 2, P, RB).transpose(0, 3, 1, 2, 4)

    combo = np.concatenate(
        [
            wsel.reshape(P, NWSEL),
            np.broadcast_to(w4il[None, :], (P, NW4)),
            np.ascontiguousarray(fjg).reshape(NCORES, P, NFJG).transpose(1, 0, 2)[
                :, 0, :
            ]
            * 0,  # placeholder, per-core below
        ],
        axis=1,
    ).astype(np.float16)
    fjg_flat = np.ascontiguousarray(fjg).reshape(NCORES, P, NFJG)

    in_maps = []
    for core in range(NCORES):
        cb = combo.copy()
        cb[:, NWSEL + NW4 :] = fjg_flat[core]
        in_maps.append({"fi": fi[core], "combo": cb})
    return in_maps


def kernel(feature_i, feature_j, mask, optical_flow, knn_inds):
    from concourse import bass_utils

    nc = _get_bass()
    in_maps = _prepare_in_maps(feature_i, feature_j, knn_inds)

    res = bass_utils.run_bass_kernel_spmd(nc, in_maps, core_ids=list(range(NCORES)))
    out = np.stack([res.results[c]["out"] for c in range(NCORES)], axis=0)
    out = out.reshape(NCORES, NF, BPC, K, G, G).transpose(0, 2, 1, 3, 4, 5)
    return np.ascontiguousarray(out.reshape(B, NF, K, G, G)).astype(np.float32)
